# revision 34
# baseline (speedup 1.0000x reference)
"""Trainium2 Bass kernel for a continuous bilinear Koopman operator rollout.

Problem (hardcoded shapes): z0 [256, 256] f32, kernel [256, 256] f32,
log_dt scalar, T=512.  Output: [256, 512, 256] f32 with
out[:, t, :] = z0 @ K_discrete^(t+1),
K_discrete = (I - 0.5*dt*K)^-1 (I + 0.5*dt*K), dt = exp(log_dt).

Strategy:
  - Host computes K_discrete (small [d,d] solve, as the sharding hint
    suggests) and a handful of its powers: A^1..A^16 and A^(16*2^i).
  - z0 and the [B, T, D] output are sharded across 8 cores along batch
    (32 trajectories per core) -- pure data parallelism.
  - On device, the T=512 serial recurrence is restructured as:
      phase B: chunk-start states sT_k = (z0 @ A^(16k)).T for k=0..31,
               built in 5 doubling rounds (s_{k+m} = s_k @ A^(16m)).
      phase C: out rows for chunk k, step j:  s_k @ A^j, j=1..16,
               as matmuls with M=128 (4 chunks x 32 batch) and N=512
               (two consecutive powers) -> PSUM -> SBUF -> 16KB-contiguous
               DMA into out[b, t, :].
  - float32r matmuls: fp32 bits in memory, single-pass PE (1 cycle/row at
    N>=256) instead of float32's LOW/HIGH double pass (4 cycles/row).
"""

import numpy as np

B = 256
D = 256
T = 512
N_CORES = 8
B_LOC = B // N_CORES      # 32
C = 16                    # chunk length (powers A^1..A^C shipped)
N_CHUNKS = T // C         # 32
N_GROUPS = N_CHUNKS // 4  # 8 groups of 4 chunks -> M=128
JP = C // 2               # 8 pairs of consecutive powers -> N=512

_CACHE = {}


def _build_bass():
    import concourse.tile as tile
    from concourse import bacc, mybir

    f32 = mybir.dt.float32
    f32r = mybir.dt.float32r
    nc = bacc.Bacc("TRN2", target_bir_lowering=False, debug=False)

    # prhs[h, :, (j-1)*256 : j*256] = A^j[h*128:(h+1)*128, :]   j=1..4
    prhs = nc.dram_tensor("prhs", [2, 128, (C // 4) * D], f32r,
                          kind="ExternalInput").ap()
    # a4t: A^4 transposed, packed lhsT-style ([r, h*256+c] = A^4[c, h*128+r])
    a4t = nc.dram_tensor("a4t", [128, 2 * D], f32r, kind="ExternalInput").ap()
    # qpow[:, 0:64]: z0 shard transposed ([r, h*32+b] = z0[b, h*128+r]);
    # qpow[:, 64 + i*512 + h*256 + c] = A^(16*2^i)[h*128 + r, c]   i=0..4
    qpow = nc.dram_tensor("qpow", [128, 64 + 5 * 2 * D], f32r,
                          kind="ExternalInput").ap()
    # a8t: A^8 transposed, packed lhsT-style ([r, h*256+c] = A^8[c, h*128+r])
    a8t = nc.dram_tensor("a8t", [128, 2 * D], f32r, kind="ExternalInput").ap()
    out = nc.dram_tensor("out", [B_LOC, T, D], f32, kind="ExternalOutput").ap()
    # out_r[k, b, j*256 + d] = out[b, 16k + j, d]
    out_r = out.rearrange("b (k j) d -> k b (j d)", j=C)

    with tile.TileContext(nc) as tc:
        with (
            tc.tile_pool(name="const", bufs=1) as cpool,
            tc.tile_pool(name="psum", bufs=8, space="PSUM") as psum_pool,
            tc.tile_pool(name="stage", bufs=8) as stage_pool,
        ):
            # Persistent SBUF tiles.
            # S[h][:, k*32 + b] = s_k[b, h*128 + d']  (chunk starts, transposed)
            S = [
                cpool.tile([128, N_CHUNKS * B_LOC], f32r, name=f"s{h}")
                for h in range(2)
            ]
            P = [cpool.tile([128, C * D], f32r, name=f"p{h}") for h in range(2)]
            Q = cpool.tile([128, 64 + 5 * 2 * D], f32r, name="q")
            A8T = cpool.tile([128, 2 * D], f32r, name="a8ts")
            A4T = cpool.tile([128, 2 * D], f32r, name="a4ts")

            from concourse.tile import add_dep_helper

            # ---- input DMAs ----
            # SP ring: z0+Q ladder slices (tiny, phase B critical path),
            # then shipped P h=0.  ACT ring: P h=1 + A4T + A8T, held behind
            # the first Q slice so the SDMA round-robin can't starve it.
            q_dmas = []
            q_dmas.append(nc.sync.dma_start(Q[:, 0:576], qpow[:, 0:576]))
            for i in range(1, 5):
                q_dmas.append(
                    nc.sync.dma_start(
                        Q[:, 64 + i * 512: 64 + (i + 1) * 512],
                        qpow[:, 64 + i * 512: 64 + (i + 1) * 512],
                    )
                )
            nc.sync.dma_start(P[0][:, 0:1024], prhs[0, :, 0:1024])
            a4_dma = nc.scalar.dma_start(A4T[:], a4t[:])
            add_dep_helper(a4_dma.ins, q_dmas[1].ins, reason="hold A4T behind Q1")
            p1 = nc.scalar.dma_start(P[1][:, 0:1024], prhs[1, :, 0:1024])
            add_dep_helper(p1.ins, q_dmas[1].ins, reason="hold P behind Q1")
            a8_dma = nc.scalar.dma_start(A8T[:], a8t[:])
            add_dep_helper(a8_dma.ins, q_dmas[1].ins, reason="hold A8T behind Q1")

            # z0t lives in Q's first 64 columns; copy into S on-chip.
            for h in range(2):
                nc.vector.tensor_copy(S[h][:, 0:B_LOC], Q[:, h * 32:(h + 1) * 32])

            # ---- emission helpers (engine streams are in-order, so the
            # emission order below is hand-interleaved to keep the first
            # output drain's dependency chain minimal) ----

            def phase_b_round(i):
                # sT_{k+m} = (A^(16m)).T @ sT_k for k < m = 2^i
                m = 1 << i
                n = B_LOC * m
                for ho in range(2):
                    ps = psum_pool.tile([128, 512], f32, name="psb", tag="ps")
                    for h in range(2):
                        nc.tensor.matmul(
                            ps[:, 0:n],
                            Q[:, 64 + i * 512 + h * D + ho * 128:
                               64 + i * 512 + h * D + (ho + 1) * 128],
                            S[h][:, 0:n],
                            start=(h == 0),
                            stop=(h == 1),
                        )
                    nc.vector.tensor_copy(S[ho][:, n:2 * n], ps[:, 0:n])

            def powers(lhsT_tile, base_j, n_i, copy_eng="vector"):
                # A^(base_j + i) = A^base_j @ A^i for i = 1..n_i
                for i in range(1, n_i + 1):
                    for ho in range(2):
                        ps = psum_pool.tile([128, 512], f32, name="psq", tag="ps")
                        for h in range(2):
                            nc.tensor.matmul(
                                ps[:, 0:256],
                                lhsT_tile[:, h * D + ho * 128:
                                          h * D + (ho + 1) * 128],
                                P[h][:, (i - 1) * 256: i * 256],
                                start=(h == 0),
                                stop=(h == 1),
                            )
                        dst = P[ho][:, (base_j + i - 1) * 256:
                                     (base_j + i) * 256]
                        if copy_eng == "scalar":
                            nc.scalar.copy(dst, ps[:, 0:256])
                        else:
                            nc.vector.tensor_copy(dst, ps[:, 0:256])

            def group(g, half):
                stage = stage_pool.tile([128, (C // 2) * D], f32, name="stage")
                for q in range(4):
                    jp = half * 4 + q
                    ps = psum_pool.tile([128, 512], f32, name="psc", tag="ps")
                    for h in range(2):
                        nc.tensor.matmul(
                            ps[:],
                            S[h][:, g * 128:(g + 1) * 128],
                            P[h][:, jp * 512:(jp + 1) * 512],
                            start=(h == 0),
                            stop=(h == 1),
                        )
                    dst = stage[:, q * 512:(q + 1) * 512]
                    if q == 1 and not (g == 0 and half == 0):
                        nc.scalar.copy(dst, ps[:])
                    else:
                        nc.vector.tensor_copy(dst, ps[:])
                # Drain: per-chunk DMAs ([32, 8, 256] view, outer dim 32 so
                # HWDGE spreads descriptors; 8KB contiguous per partition).
                # ki 0/1 (partitions 0..63, even AXI ports) on SP ring,
                # ki 2/3 (odd ports) on ACT ring.
                for ki in range(4):
                    k = 4 * g + ki
                    t0c = C * k + half * (C // 2)
                    dma_eng = nc.sync if ki < 2 else nc.scalar
                    dma_eng.dma_start(
                        out[:, t0c: t0c + C // 2, :],
                        stage[ki * B_LOC:(ki + 1) * B_LOC, :],
                    )

            # ---- emission order ----
            # Minimal chain to the first drain: rounds 0-1 (chunks 0-3),
            # A^5..8, then group 0.  Later phase-B rounds slot in just
            # before the first group that needs their chunks.
            phase_b_round(0)
            phase_b_round(1)
            powers(A4T, 4, 4)        # A^5..A^8
            group(0, 0)
            phase_b_round(2)
            group(1, 0)
            phase_b_round(3)
            group(2, 0)
            group(3, 0)
            phase_b_round(4)
            group(4, 0)
            group(5, 0)
            powers(A8T, 8, 8, copy_eng="scalar")  # A^9..A^16 (for half 1)
            group(6, 0)
            group(7, 0)
            for g in range(N_GROUPS):
                group(g, 1)

    nc.compile()
    return nc


def _host_prep(z0, kernel, log_dt):
    """fp64 host math: K_discrete and its needed powers."""
    K = np.asarray(kernel, dtype=np.float64)
    dt = float(np.exp(np.float64(np.asarray(log_dt))))
    eye = np.eye(D, dtype=np.float64)
    A = np.linalg.solve(eye - 0.5 * dt * K, eye + 0.5 * dt * K)

    pows = [None] * (C + 1)  # pows[j] = A^j
    pows[1] = A
    for j in range(2, C + 1):
        pows[j] = pows[j - 1] @ A

    # qs[i] = A^(C * 2^i), i = 0..4
    qs = [pows[C]]
    for _ in range(4):
        qs.append(qs[-1] @ qs[-1])

    # prhs [2, 128, (C//4)*D]: A^1..A^4 (A^5..16 computed on device)
    nj = C // 4
    parr = np.stack([pows[j] for j in range(1, nj + 1)], axis=0)  # [4, 256, 256]
    prhs = np.ascontiguousarray(
        parr.reshape(nj, 2, 128, D).transpose(1, 2, 0, 3).reshape(2, 128, nj * D)
    ).astype(np.float32)

    def pack_t(mat):
        # [r, h*256 + c] = mat.T[h*128 + r, c]
        mt = mat.T
        return np.ascontiguousarray(
            mt.reshape(2, 128, D).transpose(1, 0, 2).reshape(128, 2 * D)
        ).astype(np.float32)

    a4t = pack_t(pows[4])
    a8t = pack_t(pows[8])

    # qpow tail [128, 5*2*D]: [r, i*512 + h*256 + c] = qs[i][h*128 + r, c]
    qarr = np.stack(qs, axis=0)  # [5, 256, 256]
    qtail = np.ascontiguousarray(
        qarr.reshape(5, 2, 128, D).transpose(2, 0, 1, 3).reshape(128, 5 * 2 * D)
    ).astype(np.float32)

    # Per-core qpow: first 64 cols hold the core's z0 shard transposed
    # ([r, h*32 + b] = z0[b, h*128 + r]), then the shared Q ladder.
    z0 = np.asarray(z0, dtype=np.float32)
    qpows = []
    for c in range(N_CORES):
        zt = z0[c * B_LOC:(c + 1) * B_LOC, :].T  # [256, 32]
        zp = zt.reshape(2, 128, B_LOC).transpose(1, 0, 2).reshape(128, 2 * B_LOC)
        qpows.append(
            np.ascontiguousarray(np.concatenate([zp, qtail], axis=1)).astype(
                np.float32
            )
        )
    return qpows, prhs, a4t, a8t


def kernel(**inputs):
    from concourse.bass_utils import run_bass_kernel_spmd

    z0 = inputs["z0"]
    kmat = inputs["kernel"]
    log_dt = inputs["log_dt"]
    t_in = int(np.asarray(inputs["T"]))
    assert t_in == T, f"kernel hardcoded for T={T}, got {t_in}"
    assert tuple(np.shape(z0)) == (B, D)

    qpows, prhs, a4t, a8t = _host_prep(z0, kmat, log_dt)

    if "nc" not in _CACHE:
        _CACHE["nc"] = _build_bass()
    nc = _CACHE["nc"]

    in_maps = [
        {"prhs": prhs, "qpow": qpows[c], "a4t": a4t, "a8t": a8t}
        for c in range(N_CORES)
    ]
    res = run_bass_kernel_spmd(nc, in_maps, core_ids=list(range(N_CORES)))
    return np.concatenate([res.results[c]["out"] for c in range(N_CORES)], axis=0)
